# revision 41
# baseline (speedup 1.0000x reference)
"""Trainium2 Bass kernel for nn_AttentionPromptExtrapolation.

Reference computation (B,N,P,D,K = 32,512,25,128,64):
    keep[n,p] = (n not in s_mti) and (p != 24)            # {0,1}, same for all b
    su = sigmoid(patches @ u.T);  su *= (su>0.5) * keep
    sm = sigmoid(patches @ m.T);  sm *= (sm>0.5) * (1-keep)
    out = patches + su @ u + sm @ m

Key observation: each row (b,n,p) uses exactly ONE of the two prompt tables
(u if keep, m otherwise). The host permutes rows so all keep-rows come
first (100 sub-blocks of 512 rows per core, no padding — the one sub-block
containing the group boundary splits its matmuls at the boundary column).
Each span needs a single K=64 score matmul against its table — no masking
on the device at all:

    z  = x_block @ T.T          [64, 512]  (T = u or m by group)
    st = (z > 0) * sigmoid(z)
    out_block = x_block + st.T @ T

The kernel is HBM-bandwidth bound, so all DRAM I/O is fp16: the host ships
patches TRANSPOSED ([D, rows] row-major) AND cast to fp16, and the output
goes back as fp16 (tolerance is rel 2e-2; fp16 costs ~3e-3 from borderline
threshold flips where z ~ 0). That halves traffic vs fp32 (26 MB/core) and
makes every matmul a full-rate fp16 op (1 col/cycle vs 4 for fp32).

Per-pair pipeline (2 sub-blocks per PSUM bank via tile_position packing):
score matmuls -> ACT sigmoid -> DVE (z>0)*sig -> [one pair behind] identity
matmul folds "+ x" into the output PSUM bank, score@T accumulates, then the
output leaves PSUM via one of three per-sub-block paths: ACT fp16 downcast
(50%), DVE fp16 downcast (30%), or a direct DVE x+y add with no identity
matmul (20%) — ACT and DVE are the only engines that can read PSUM, and the
add share is sized so PE work drops ~7% while the PE stays >90% busy. That
matters because a lightly loaded PE falls out of its 2.4 GHz p-state and
every matmul on the critical path doubles (measured on three variants).
x megablocks stream in on the SP queue with the constants on the ACT queue
so neither serializes the other at ramp; out-DMAs issue from the ACT queue.
"""

import numpy as np

import concourse.bacc as bacc
import concourse.tile as tile
from concourse import mybir
from concourse.alu_op_type import AluOpType

B, N, P, D, K = 32, 512, 25, 128, 64
K2 = 2 * K              # 128
NCORES = 8
BPC = B // NCORES       # batches per core = 4
NP = N * P              # rows per batch = 12800
BLK = 512               # rows per compute sub-block
NBLOCKS = 100           # sub-blocks per core (50 pairs, no padding)
MB = 8                  # steady-state sub-blocks per megablock DMA
ROWS = NBLOCKS * BLK    # 51200 rows per core
T_MTI = 24
SIG_CUT = 0.0

F32 = mybir.dt.float32
F16 = mybir.dt.float16


def build_nc(cut):
    """Build the single-core bass program. Rows [0, cut) are keep-group (use
    table u = C[0:64]); [cut, ROWS) are masked-group (m = C[64:128]). The one
    sub-block containing `cut` splits its matmuls at the boundary column, so
    no padding rows are needed."""
    nc = bacc.Bacc(None, target_bir_lowering=False)
    b0, off = divmod(cut, BLK)

    x_d = nc.dram_tensor("x", [D, ROWS], F16, kind="ExternalInput")       # x.T
    ct_d = nc.dram_tensor("ct", [D, K2], F16, kind="ExternalInput")       # C.T
    # C fp16 replicated in both partition halves: [u | m] on parts 0:64
    # and again on parts 64:128 (mm2's contraction partitions must match
    # whichever half of st it consumes)
    cb_d = nc.dram_tensor("cboth", [K2, 2 * D], F16, kind="ExternalInput")
    id_d = nc.dram_tensor("ident", [D, D], F16, kind="ExternalInput")
    out_d = nc.dram_tensor("out", [D, ROWS], F16, kind="ExternalOutput")  # out.T

    def spans(s):
        # [(col_lo, col_hi, group)] covering sub-block s's 512 columns
        if s < b0:
            return [(0, BLK, 0)]
        if s > b0 or off == 0:
            return [(0, BLK, 1)]
        return [(0, off, 0), (off, BLK, 1)]

    with tile.TileContext(nc) as tc:
        with (
            tc.tile_pool(name="consts", bufs=1) as consts,
            tc.tile_pool(name="xp", bufs=5) as xp,
            tc.tile_pool(name="sgp", bufs=8) as sgp,
            tc.tile_pool(name="op", bufs=4) as op,
            tc.tile_pool(name="ps_z", bufs=2, space="PSUM") as ps_z,
            tc.tile_pool(name="ps_y", bufs=6, space="PSUM") as ps_y,
        ):
            # consts go out on the ACT queue so they don't serialize ahead
            # of the x stream on the SP queue (each issue costs ~600ns of
            # sequencer time; the first x megablock was landing ~3us late)
            ct_sb = consts.tile([D, K2], F16)
            nc.scalar.dma_start(ct_sb, ct_d[:, :])
            cb_sb = consts.tile([K2, 2 * D], F16)
            nc.scalar.dma_start(cb_sb, cb_d[:, :])
            id_sb = consts.tile([D, D], F16)
            nc.scalar.dma_start(id_sb, id_d[:, :])

            # preload the sigmoid ACT table set (~2.7us) while the first x
            # megablock is still streaming, instead of on the critical path
            warm_sb = consts.tile([128, 1], F16)
            nc.scalar.activation(
                warm_sb, ct_sb[:, 0:1], mybir.ActivationFunctionType.Sigmoid
            )

            # small first/last megablocks so the pipeline ramps and drains fast
            sizes = [2, 4] + [MB] * 11 + [4, 2]
            assert sum(sizes) == NBLOCKS
            pends = []  # deque of (st_sb, x_mb, o_mb, base, lp, sz, rowoff)

            def stage1(pend):
                # identity matmuls one pair behind: their only waits (free
                # PSUM bank, old x) are satisfied at dispatch, so they
                # pipeline tightly behind the score matmuls. ALL pairs get
                # identity matmuls here — PE density ~300 matmuls holds the
                # 2.4 GHz p-state (the session-4 stagger with 20% add-class
                # pairs dropped PE to 62us busy and the clock sagged)
                st_sb, px_mb, po_mb, base, lp, sz, rowoff = pend[0]
                pair_id = (base + 2 * lp) // 2
                cls = "AC"[pair_id % 2]
                y_tiles = []
                for slot in range(2):
                    y_ps = ps_y.tile([128, BLK], F32)
                    y_tiles.append(y_ps)
                    msub = 2 * lp + slot
                    nc.tensor.matmul(
                        y_ps,
                        lhsT=id_sb,
                        rhs=px_mb[:, msub * BLK:(msub + 1) * BLK],
                        start=True,
                        stop=False,
                        skip_group_check=True,
                    )
                pend[1] = (cls, y_tiles)

            def stage2(pend):
                # st-consuming matmuls TWO pairs behind: st is two
                # iterations old, so the PE never stalls on the ~1.3us
                # z->sigmoid->threshold chain (8-16 PE gaps of 0.5-1.1us at
                # depth 1). Unlike a monolithic depth-2 flush, these waits
                # are satisfied at dispatch and don't clog the 4-deep
                # engine wait queues.
                st_sb, px_mb, po_mb, base, lp, sz, rowoff = pend[0]
                cls, y_tiles = pend[1]
                for slot in range(2):
                    msub = 2 * lp + slot
                    for lo, hi, g in spans(base + msub):
                        nc.tensor.matmul(
                            y_tiles[slot][:, lo:hi],
                            lhsT=cb_sb[slot * K:(slot + 1) * K,
                                       g * D:(g + 1) * D],
                            rhs=st_sb[slot * K:(slot + 1) * K, lo:hi],
                            start=False,
                            stop=True,
                            tile_position=(slot * K, 0),
                            skip_group_check=True,
                        )
                for slot in range(2):
                    msub = 2 * lp + slot
                    dst = po_mb[:, msub * BLK:(msub + 1) * BLK]
                    if cls == 'A':
                        nc.scalar.copy(dst, y_tiles[slot])
                    else:
                        nc.vector.tensor_copy(dst, y_tiles[slot])
                if lp == sz // 2 - 1:
                    nc.scalar.dma_start(
                        out_d[:, rowoff:rowoff + sz * BLK], po_mb
                    )

            base = 0
            for sz in sizes:
                rowoff = base * BLK
                x_mb = xp.tile([128, sz * BLK], F16, tag="x_mb")
                if base == 0:
                    # per-block loads so the very first matmul starts sooner
                    for q in range(sz):
                        nc.sync.dma_start(
                            x_mb[:, q * BLK:(q + 1) * BLK],
                            x_d[:, rowoff + q * BLK:rowoff + (q + 1) * BLK],
                        )
                else:
                    nc.sync.dma_start(x_mb, x_d[:, rowoff:rowoff + sz * BLK])
                o_mb = op.tile([128, sz * BLK], F16, tag="o_mb")
                for lp in range(sz // 2):
                    z_ps = ps_z.tile([128, BLK], F32)
                    for slot in range(2):
                        msub = 2 * lp + slot
                        # z[slot half] [64, 512] = T_g @ x_sub (the boundary
                        # sub-block splits at the keep/masked column)
                        for lo, hi, g in spans(base + msub):
                            nc.tensor.matmul(
                                z_ps[slot * K:(slot + 1) * K, lo:hi],
                                lhsT=ct_sb[:, g * K:(g + 1) * K],
                                rhs=x_mb[:, msub * BLK + lo:msub * BLK + hi],
                                start=True,
                                stop=True,
                                tile_position=(0, slot * K),
                            )

                    sig_sb = sgp.tile([128, BLK], F16)
                    nc.scalar.activation(
                        sig_sb, z_ps, mybir.ActivationFunctionType.Sigmoid
                    )
                    # st = (z > cut) * sigmoid(z), both packed sub-blocks
                    st_sb = sgp.tile([128, BLK], F16)
                    nc.vector.scalar_tensor_tensor(
                        out=st_sb,
                        in0=z_ps,
                        scalar=SIG_CUT,
                        in1=sig_sb,
                        op0=AluOpType.is_gt,
                        op1=AluOpType.mult,
                    )

                    # staggered pipeline: identity matmuls one pair
                    # behind, st-consuming matmuls + output ops two behind
                    if pends:
                        stage1(pends[-1])
                    if len(pends) >= 2:
                        stage2(pends.pop(0))
                    pends.append([(st_sb, x_mb, o_mb, base, lp, sz, rowoff),
                                  None])
                base += sz

            for pend in pends:
                if pend[1] is None:
                    stage1(pend)
                stage2(pend)

    nc.compile()
    return nc


def plan_permutation(s_mti):
    """Row permutation grouping keep-rows first (no padding).
    Returns (perm, cut) with cut = number of keep rows."""
    n_mask = np.ones(N, np.float32)
    n_mask[np.asarray(s_mti)] = 0.0
    t_mask = np.ones(P, np.float32)
    t_mask[T_MTI] = 0.0
    keep = (n_mask[:, None] * t_mask[None, :]).reshape(-1)   # [NP]
    keep_core = np.tile(keep, BPC)                           # [BPC*NP]
    idx_keep = np.flatnonzero(keep_core == 1.0)
    idx_masked = np.flatnonzero(keep_core == 0.0)
    perm = np.concatenate([idx_keep, idx_masked])
    return perm, len(idx_keep)


def host_inputs(patches, u_prompt, m_prompt, s_mti):
    patches = np.asarray(patches, dtype=np.float32)
    u = np.asarray(u_prompt, dtype=np.float32)
    m = np.asarray(m_prompt, dtype=np.float32)

    C = np.concatenate([u, m], axis=0)                       # [128, 128]
    cf = C.astype(np.float16)
    ct = np.ascontiguousarray(cf.T)                          # [D, 2K] f16
    cboth = np.ascontiguousarray(
        np.concatenate([np.concatenate([cf[:K], cf[K:]], 1)] * 2, 0)
    )                                                        # [128, 256]
    ident = np.eye(D, dtype=np.float16)

    perm, cut = plan_permutation(s_mti)

    x_flat = patches.astype(np.float16).reshape(B, NP, D)
    in_maps = []
    for c in range(NCORES):
        xT = x_flat[c * BPC:(c + 1) * BPC].reshape(BPC * NP, D).T  # [D, rows]
        xs = np.ascontiguousarray(xT[:, perm])
        in_maps.append({"x": xs, "ct": ct, "cboth": cboth, "ident": ident})
    return in_maps, (perm, cut)


_NC_CACHE = {}


def kernel(patches, u_prompt, m_prompt, s_mti, s_uti=None, trace=False, **kw):
    from concourse.bass_utils import run_bass_kernel_spmd

    in_maps, (perm, cut) = host_inputs(patches, u_prompt, m_prompt, s_mti)

    if cut not in _NC_CACHE:
        _NC_CACHE[cut] = build_nc(cut)
    nc = _NC_CACHE[cut]

    res = run_bass_kernel_spmd(nc, in_maps, list(range(NCORES)), trace=trace)
    out = np.empty((B, NP, D), np.float32)
    for c in range(NCORES):
        oT = res.results[c]["out"]                           # [D, ROWS] f16
        dst = out[c * BPC:(c + 1) * BPC].reshape(BPC * NP, D)
        dst[perm] = oT.T
    out = out.reshape(B, N, P, D)
    if trace:
        kernel.last_results = res
    return out


# revision 42
# speedup vs baseline: 1.0457x; 1.0457x over previous
"""Trainium2 Bass kernel for nn_AttentionPromptExtrapolation.

Reference computation (B,N,P,D,K = 32,512,25,128,64):
    keep[n,p] = (n not in s_mti) and (p != 24)            # {0,1}, same for all b
    su = sigmoid(patches @ u.T);  su *= (su>0.5) * keep
    sm = sigmoid(patches @ m.T);  sm *= (sm>0.5) * (1-keep)
    out = patches + su @ u + sm @ m

Key observation: each row (b,n,p) uses exactly ONE of the two prompt tables
(u if keep, m otherwise). The host permutes rows so all keep-rows come
first (100 sub-blocks of 512 rows per core, no padding — the one sub-block
containing the group boundary splits its matmuls at the boundary column).
Each span needs a single K=64 score matmul against its table — no masking
on the device at all:

    z  = x_block @ T.T          [64, 512]  (T = u or m by group)
    st = (z > 0) * sigmoid(z)
    out_block = x_block + st.T @ T

The kernel is HBM-bandwidth bound, so all DRAM I/O is fp16: the host ships
patches TRANSPOSED ([D, rows] row-major) AND cast to fp16, and the output
goes back as fp16 (tolerance is rel 2e-2; fp16 costs ~3e-3 from borderline
threshold flips where z ~ 0). That halves traffic vs fp32 (26 MB/core) and
makes every matmul a full-rate fp16 op (1 col/cycle vs 4 for fp32).

Per-pair pipeline (2 sub-blocks per PSUM bank via tile_position packing):
score matmuls -> ACT sigmoid -> DVE (z>0)*sig -> [one pair behind] identity
matmul folds "+ x" into the output PSUM bank, score@T accumulates, then the
output leaves PSUM via one of three per-sub-block paths: ACT fp16 downcast
(50%), DVE fp16 downcast (30%), or a direct DVE x+y add with no identity
matmul (20%) — ACT and DVE are the only engines that can read PSUM, and the
add share is sized so PE work drops ~7% while the PE stays >90% busy. That
matters because a lightly loaded PE falls out of its 2.4 GHz p-state and
every matmul on the critical path doubles (measured on three variants).
x megablocks stream in on the SP queue with the constants on the ACT queue
so neither serializes the other at ramp; out-DMAs issue from the ACT queue.
"""

import numpy as np

import concourse.bacc as bacc
import concourse.tile as tile
from concourse import mybir
from concourse.alu_op_type import AluOpType

B, N, P, D, K = 32, 512, 25, 128, 64
K2 = 2 * K              # 128
NCORES = 8
BPC = B // NCORES       # batches per core = 4
NP = N * P              # rows per batch = 12800
BLK = 512               # rows per compute sub-block
NBLOCKS = 100           # sub-blocks per core (50 pairs, no padding)
MB = 8                  # steady-state sub-blocks per megablock DMA
ROWS = NBLOCKS * BLK    # 51200 rows per core
T_MTI = 24
SIG_CUT = 0.0

F32 = mybir.dt.float32
F16 = mybir.dt.float16


def build_nc(cut):
    """Build the single-core bass program. Rows [0, cut) are keep-group (use
    table u = C[0:64]); [cut, ROWS) are masked-group (m = C[64:128]). The one
    sub-block containing `cut` splits its matmuls at the boundary column, so
    no padding rows are needed."""
    nc = bacc.Bacc(None, target_bir_lowering=False)
    b0, off = divmod(cut, BLK)

    x_d = nc.dram_tensor("x", [D, ROWS], F16, kind="ExternalInput")       # x.T
    ct_d = nc.dram_tensor("ct", [D, K2], F16, kind="ExternalInput")       # C.T
    # C fp16 replicated in both partition halves: [u | m] on parts 0:64
    # and again on parts 64:128 (mm2's contraction partitions must match
    # whichever half of st it consumes)
    cb_d = nc.dram_tensor("cboth", [K2, 2 * D], F16, kind="ExternalInput")
    id_d = nc.dram_tensor("ident", [D, D], F16, kind="ExternalInput")
    out_d = nc.dram_tensor("out", [D, ROWS], F16, kind="ExternalOutput")  # out.T

    def spans(s):
        # [(col_lo, col_hi, group)] covering sub-block s's 512 columns
        if s < b0:
            return [(0, BLK, 0)]
        if s > b0 or off == 0:
            return [(0, BLK, 1)]
        return [(0, off, 0), (off, BLK, 1)]

    with tile.TileContext(nc) as tc:
        with (
            tc.tile_pool(name="consts", bufs=1) as consts,
            tc.tile_pool(name="xp", bufs=5) as xp,
            tc.tile_pool(name="sgp", bufs=8) as sgp,
            tc.tile_pool(name="op", bufs=4) as op,
            tc.tile_pool(name="ps_z", bufs=3, space="PSUM") as ps_z,
            tc.tile_pool(name="ps_y", bufs=4, space="PSUM") as ps_y,
        ):
            # consts go out on the ACT queue so they don't serialize ahead
            # of the x stream on the SP queue (each issue costs ~600ns of
            # sequencer time; the first x megablock was landing ~3us late)
            ct_sb = consts.tile([D, K2], F16)
            nc.scalar.dma_start(ct_sb, ct_d[:, :])
            cb_sb = consts.tile([K2, 2 * D], F16)
            nc.scalar.dma_start(cb_sb, cb_d[:, :])
            id_sb = consts.tile([D, D], F16)
            nc.scalar.dma_start(id_sb, id_d[:, :])

            # preload the sigmoid ACT table set (~2.7us) while the first x
            # megablock is still streaming, instead of on the critical path
            warm_sb = consts.tile([128, 1], F16)
            nc.scalar.activation(
                warm_sb, ct_sb[:, 0:1], mybir.ActivationFunctionType.Sigmoid
            )

            # small first/last megablocks so the pipeline ramps and drains fast
            sizes = [2, 4] + [MB] * 11 + [4, 2]
            assert sum(sizes) == NBLOCKS
            pends = []  # deque of (st_sb, x_mb, o_mb, base, lp, sz, rowoff)

            def flush(pend):
                st_sb, px_mb, po_mb, base, lp, sz, rowoff = pend
                # both identity matmuls first (they only wait on a free PSUM
                # bank, so they pipeline tightly behind the score matmuls),
                # then both score@T matmuls (one semaphore wait on st for
                # the group instead of interleaved waits — exposed PE
                # pipeline drains cost ~110ns per waiting matmul)
                # output classes: even sub-blocks downcast on ACT, odd on
                # DVE; 2 of every 5 DVE sub-blocks skip the identity matmul
                # and do a real x+y add instead — trims PE work ~7% (it is
                # the pacer) while keeping it >90% busy so the 2.4 GHz
                # p-state holds
                def odc(m):
                    if m % 2 == 0:
                        return 'A'
                    return 'V' if (m // 2) % 5 in (1, 3) else 'C'
                y_tiles = []
                for slot in range(2):
                    msub = 2 * lp + slot
                    y_ps = ps_y.tile([128, BLK], F32)
                    y_tiles.append(y_ps)
                    if odc(base + msub) == 'V':
                        continue
                    # out = x + st.T @ T accumulated on the PE: identity
                    # matmul writes x (start), score matmul adds on top
                    nc.tensor.matmul(
                        y_ps,
                        lhsT=id_sb,
                        rhs=px_mb[:, msub * BLK:(msub + 1) * BLK],
                        start=True,
                        stop=False,
                        skip_group_check=True,
                    )
                for slot in range(2):
                    msub = 2 * lp + slot
                    first = odc(base + msub) == 'V'
                    for lo, hi, g in spans(base + msub):
                        nc.tensor.matmul(
                            y_tiles[slot][:, lo:hi],
                            lhsT=cb_sb[slot * K:(slot + 1) * K,
                                       g * D:(g + 1) * D],
                            rhs=st_sb[slot * K:(slot + 1) * K, lo:hi],
                            start=first,
                            stop=True,
                            tile_position=(slot * K, 0),
                            skip_group_check=True,
                        )
                for slot in range(2):
                    msub = 2 * lp + slot
                    dst = po_mb[:, msub * BLK:(msub + 1) * BLK]
                    cls = odc(base + msub)
                    if cls == 'A':
                        nc.scalar.copy(dst, y_tiles[slot])
                    elif cls == 'C':
                        nc.vector.tensor_copy(dst, y_tiles[slot])
                    else:
                        nc.vector.tensor_tensor(
                            out=dst,
                            in0=px_mb[:, msub * BLK:(msub + 1) * BLK],
                            in1=y_tiles[slot],
                            op=AluOpType.add,
                        )
                if lp == sz // 2 - 1:
                    nc.scalar.dma_start(
                        out_d[:, rowoff:rowoff + sz * BLK], po_mb
                    )

            base = 0
            for sz in sizes:
                rowoff = base * BLK
                x_mb = xp.tile([128, sz * BLK], F16, tag="x_mb")
                if base == 0:
                    # per-block loads so the very first matmul starts sooner
                    for q in range(sz):
                        nc.sync.dma_start(
                            x_mb[:, q * BLK:(q + 1) * BLK],
                            x_d[:, rowoff + q * BLK:rowoff + (q + 1) * BLK],
                        )
                else:
                    nc.sync.dma_start(x_mb, x_d[:, rowoff:rowoff + sz * BLK])
                o_mb = op.tile([128, sz * BLK], F16, tag="o_mb")
                for lp in range(sz // 2):
                    z_ps = ps_z.tile([128, BLK], F32)
                    for slot in range(2):
                        msub = 2 * lp + slot
                        # z[slot half] [64, 512] = T_g @ x_sub (the boundary
                        # sub-block splits at the keep/masked column)
                        for lo, hi, g in spans(base + msub):
                            nc.tensor.matmul(
                                z_ps[slot * K:(slot + 1) * K, lo:hi],
                                lhsT=ct_sb[:, g * K:(g + 1) * K],
                                rhs=x_mb[:, msub * BLK + lo:msub * BLK + hi],
                                start=True,
                                stop=True,
                                tile_position=(0, slot * K),
                            )

                    sig_sb = sgp.tile([128, BLK], F16)
                    nc.scalar.activation(
                        sig_sb, z_ps, mybir.ActivationFunctionType.Sigmoid
                    )
                    # st = (z > cut) * sigmoid(z), both packed sub-blocks
                    st_sb = sgp.tile([128, BLK], F16)
                    nc.vector.scalar_tensor_tensor(
                        out=st_sb,
                        in0=z_ps,
                        scalar=SIG_CUT,
                        in1=sig_sb,
                        op0=AluOpType.is_gt,
                        op1=AluOpType.mult,
                    )

                    # second matmul / copy / store run one pair behind so
                    # the PE never waits on the current pair's sigmoid/STT
                    # (depth 2 measured worse: 100.0us vs 98.4us)
                    pends.append((st_sb, x_mb, o_mb, base, lp, sz, rowoff))
                    if len(pends) > 1:
                        flush(pends.pop(0))
                base += sz

            for p in pends:
                flush(p)

    nc.compile()
    return nc


def plan_permutation(s_mti):
    """Row permutation grouping keep-rows first (no padding).
    Returns (perm, cut) with cut = number of keep rows."""
    n_mask = np.ones(N, np.float32)
    n_mask[np.asarray(s_mti)] = 0.0
    t_mask = np.ones(P, np.float32)
    t_mask[T_MTI] = 0.0
    keep = (n_mask[:, None] * t_mask[None, :]).reshape(-1)   # [NP]
    keep_core = np.tile(keep, BPC)                           # [BPC*NP]
    idx_keep = np.flatnonzero(keep_core == 1.0)
    idx_masked = np.flatnonzero(keep_core == 0.0)
    perm = np.concatenate([idx_keep, idx_masked])
    return perm, len(idx_keep)


def host_inputs(patches, u_prompt, m_prompt, s_mti):
    patches = np.asarray(patches, dtype=np.float32)
    u = np.asarray(u_prompt, dtype=np.float32)
    m = np.asarray(m_prompt, dtype=np.float32)

    C = np.concatenate([u, m], axis=0)                       # [128, 128]
    cf = C.astype(np.float16)
    ct = np.ascontiguousarray(cf.T)                          # [D, 2K] f16
    cboth = np.ascontiguousarray(
        np.concatenate([np.concatenate([cf[:K], cf[K:]], 1)] * 2, 0)
    )                                                        # [128, 256]
    ident = np.eye(D, dtype=np.float16)

    perm, cut = plan_permutation(s_mti)

    x_flat = patches.astype(np.float16).reshape(B, NP, D)
    in_maps = []
    for c in range(NCORES):
        xT = x_flat[c * BPC:(c + 1) * BPC].reshape(BPC * NP, D).T  # [D, rows]
        xs = np.ascontiguousarray(xT[:, perm])
        in_maps.append({"x": xs, "ct": ct, "cboth": cboth, "ident": ident})
    return in_maps, (perm, cut)


_NC_CACHE = {}


def kernel(patches, u_prompt, m_prompt, s_mti, s_uti=None, trace=False, **kw):
    from concourse.bass_utils import run_bass_kernel_spmd

    in_maps, (perm, cut) = host_inputs(patches, u_prompt, m_prompt, s_mti)

    if cut not in _NC_CACHE:
        _NC_CACHE[cut] = build_nc(cut)
    nc = _NC_CACHE[cut]

    res = run_bass_kernel_spmd(nc, in_maps, list(range(NCORES)), trace=trace)
    out = np.empty((B, NP, D), np.float32)
    for c in range(NCORES):
        oT = res.results[c]["out"]                           # [D, ROWS] f16
        dst = out[c * BPC:(c + 1) * BPC].reshape(BPC * NP, D)
        dst[perm] = oT.T
    out = out.reshape(B, N, P, D)
    if trace:
        kernel.last_results = res
    return out


# revision 43
# speedup vs baseline: 1.0896x; 1.0420x over previous
"""Trainium2 Bass kernel for nn_AttentionPromptExtrapolation.

Reference computation (B,N,P,D,K = 32,512,25,128,64):
    keep[n,p] = (n not in s_mti) and (p != 24)            # {0,1}, same for all b
    su = sigmoid(patches @ u.T);  su *= (su>0.5) * keep
    sm = sigmoid(patches @ m.T);  sm *= (sm>0.5) * (1-keep)
    out = patches + su @ u + sm @ m

Key observation: each row (b,n,p) uses exactly ONE of the two prompt tables
(u if keep, m otherwise). The host permutes rows so all keep-rows come
first (100 sub-blocks of 512 rows per core, no padding — the one sub-block
containing the group boundary splits its matmuls at the boundary column).
Each span needs a single K=64 score matmul against its table — no masking
on the device at all:

    z  = x_block @ T.T          [64, 512]  (T = u or m by group)
    st = (z > 0) * sigmoid(z)
    out_block = x_block + st.T @ T

The kernel is HBM-bandwidth bound, so all DRAM I/O is fp16: the host ships
patches TRANSPOSED ([D, rows] row-major) AND cast to fp16, and the output
goes back as fp16 (tolerance is rel 2e-2; fp16 costs ~3e-3 from borderline
threshold flips where z ~ 0). That halves traffic vs fp32 (26 MB/core) and
makes every matmul a full-rate fp16 op (1 col/cycle vs 4 for fp32).

Per-pair pipeline (2 sub-blocks per PSUM bank via tile_position packing):
score matmuls -> ACT sigmoid -> DVE (z>0)*sig -> [one pair behind] identity
matmul folds "+ x" into the output PSUM bank, score@T accumulates, then the
output leaves PSUM via one of three per-sub-block paths: ACT fp16 downcast
(50%), DVE fp16 downcast (30%), or a direct DVE x+y add with no identity
matmul (20%) — ACT and DVE are the only engines that can read PSUM, and the
add share is sized so PE work drops ~7% while the PE stays >90% busy. That
matters because a lightly loaded PE falls out of its 2.4 GHz p-state and
every matmul on the critical path doubles (measured on three variants).
x megablocks stream in on the SP queue with the constants on the ACT queue
so neither serializes the other at ramp; out-DMAs issue from the ACT queue.
"""

import numpy as np

import concourse.bacc as bacc
import concourse.tile as tile
from concourse import mybir
from concourse.alu_op_type import AluOpType

B, N, P, D, K = 32, 512, 25, 128, 64
K2 = 2 * K              # 128
NCORES = 8
BPC = B // NCORES       # batches per core = 4
NP = N * P              # rows per batch = 12800
BLK = 512               # rows per compute sub-block
NBLOCKS = 100           # sub-blocks per core (50 pairs, no padding)
MB = 8                  # steady-state sub-blocks per megablock DMA
ROWS = NBLOCKS * BLK    # 51200 rows per core
T_MTI = 24
SIG_CUT = 0.0

F32 = mybir.dt.float32
F16 = mybir.dt.float16


def build_nc(cut):
    """Build the single-core bass program. Rows [0, cut) are keep-group (use
    table u = C[0:64]); [cut, ROWS) are masked-group (m = C[64:128]). The one
    sub-block containing `cut` splits its matmuls at the boundary column, so
    no padding rows are needed."""
    nc = bacc.Bacc(None, target_bir_lowering=False)
    b0, off = divmod(cut, BLK)

    x_d = nc.dram_tensor("x", [D, ROWS], F16, kind="ExternalInput")       # x.T
    ct_d = nc.dram_tensor("ct", [D, K2], F16, kind="ExternalInput")       # C.T
    # C fp16 replicated in both partition halves: [u | m] on parts 0:64
    # and again on parts 64:128 (mm2's contraction partitions must match
    # whichever half of st it consumes)
    cb_d = nc.dram_tensor("cboth", [K2, 2 * D], F16, kind="ExternalInput")
    id_d = nc.dram_tensor("ident", [D, D], F16, kind="ExternalInput")
    out_d = nc.dram_tensor("out", [D, ROWS], F16, kind="ExternalOutput")  # out.T

    def spans(s):
        # [(col_lo, col_hi, group)] covering sub-block s's 512 columns
        if s < b0:
            return [(0, BLK, 0)]
        if s > b0 or off == 0:
            return [(0, BLK, 1)]
        return [(0, off, 0), (off, BLK, 1)]

    with tile.TileContext(nc) as tc:
        with (
            tc.tile_pool(name="consts", bufs=1) as consts,
            tc.tile_pool(name="xp", bufs=5) as xp,
            tc.tile_pool(name="sgp", bufs=8) as sgp,
            tc.tile_pool(name="op", bufs=4) as op,
            tc.tile_pool(name="ps_z", bufs=3, space="PSUM") as ps_z,
            tc.tile_pool(name="ps_y", bufs=5, space="PSUM") as ps_y,
        ):
            # consts go out on the ACT queue so they don't serialize ahead
            # of the x stream on the SP queue (each issue costs ~600ns of
            # sequencer time; the first x megablock was landing ~3us late)
            ct_sb = consts.tile([D, K2], F16)
            nc.scalar.dma_start(ct_sb, ct_d[:, :])
            cb_sb = consts.tile([K2, 2 * D], F16)
            nc.scalar.dma_start(cb_sb, cb_d[:, :])
            id_sb = consts.tile([D, D], F16)
            nc.scalar.dma_start(id_sb, id_d[:, :])

            # preload the sigmoid ACT table set (~2.7us) while the first x
            # megablock is still streaming, instead of on the critical path
            warm_sb = consts.tile([128, 1], F16)
            nc.scalar.activation(
                warm_sb, ct_sb[:, 0:1], mybir.ActivationFunctionType.Sigmoid
            )

            # small first/last megablocks so the pipeline ramps and drains fast
            sizes = [2, 4] + [MB] * 11 + [4, 2]
            assert sum(sizes) == NBLOCKS
            pends = []  # deque of (st_sb, x_mb, o_mb, base, lp, sz, rowoff)

            def flush(pend):
                st_sb, px_mb, po_mb, base, lp, sz, rowoff = pend
                # both identity matmuls first (they only wait on a free PSUM
                # bank, so they pipeline tightly behind the score matmuls),
                # then both score@T matmuls (one semaphore wait on st for
                # the group instead of interleaved waits — exposed PE
                # pipeline drains cost ~110ns per waiting matmul)
                # output classes: even sub-blocks downcast on ACT, odd on
                # DVE; 2 of every 5 DVE sub-blocks skip the identity matmul
                # and do a real x+y add instead — trims PE work ~7% (it is
                # the pacer) while keeping it >90% busy so the 2.4 GHz
                # p-state holds
                def odc(m):
                    if m % 2 == 0:
                        return 'A'
                    return 'V' if (m // 2) % 5 in (1, 3) else 'C'
                y_tiles = []
                for slot in range(2):
                    msub = 2 * lp + slot
                    y_ps = ps_y.tile([128, BLK], F32)
                    y_tiles.append(y_ps)
                    if odc(base + msub) == 'V':
                        continue
                    # out = x + st.T @ T accumulated on the PE: identity
                    # matmul writes x (start), score matmul adds on top
                    nc.tensor.matmul(
                        y_ps,
                        lhsT=id_sb,
                        rhs=px_mb[:, msub * BLK:(msub + 1) * BLK],
                        start=True,
                        stop=False,
                        skip_group_check=True,
                    )
                for slot in range(2):
                    msub = 2 * lp + slot
                    first = odc(base + msub) == 'V'
                    for lo, hi, g in spans(base + msub):
                        nc.tensor.matmul(
                            y_tiles[slot][:, lo:hi],
                            lhsT=cb_sb[slot * K:(slot + 1) * K,
                                       g * D:(g + 1) * D],
                            rhs=st_sb[slot * K:(slot + 1) * K, lo:hi],
                            start=first,
                            stop=True,
                            tile_position=(slot * K, 0),
                            skip_group_check=True,
                        )
                for slot in range(2):
                    msub = 2 * lp + slot
                    dst = po_mb[:, msub * BLK:(msub + 1) * BLK]
                    cls = odc(base + msub)
                    if cls == 'A':
                        nc.scalar.copy(dst, y_tiles[slot])
                    elif cls == 'C':
                        nc.vector.tensor_copy(dst, y_tiles[slot])
                    else:
                        nc.vector.tensor_tensor(
                            out=dst,
                            in0=px_mb[:, msub * BLK:(msub + 1) * BLK],
                            in1=y_tiles[slot],
                            op=AluOpType.add,
                        )
                if lp == sz // 2 - 1:
                    nc.scalar.dma_start(
                        out_d[:, rowoff:rowoff + sz * BLK], po_mb
                    )

            base = 0
            for sz in sizes:
                rowoff = base * BLK
                x_mb = xp.tile([128, sz * BLK], F16, tag="x_mb")
                if base == 0:
                    # per-block loads so the very first matmul starts sooner
                    for q in range(sz):
                        nc.sync.dma_start(
                            x_mb[:, q * BLK:(q + 1) * BLK],
                            x_d[:, rowoff + q * BLK:rowoff + (q + 1) * BLK],
                        )
                else:
                    nc.sync.dma_start(x_mb, x_d[:, rowoff:rowoff + sz * BLK])
                o_mb = op.tile([128, sz * BLK], F16, tag="o_mb")
                for lp in range(sz // 2):
                    z_ps = ps_z.tile([128, BLK], F32)
                    for slot in range(2):
                        msub = 2 * lp + slot
                        # z[slot half] [64, 512] = T_g @ x_sub (the boundary
                        # sub-block splits at the keep/masked column)
                        for lo, hi, g in spans(base + msub):
                            nc.tensor.matmul(
                                z_ps[slot * K:(slot + 1) * K, lo:hi],
                                lhsT=ct_sb[:, g * K:(g + 1) * K],
                                rhs=x_mb[:, msub * BLK + lo:msub * BLK + hi],
                                start=True,
                                stop=True,
                                tile_position=(0, slot * K),
                            )

                    sig_sb = sgp.tile([128, BLK], F16)
                    nc.scalar.activation(
                        sig_sb, z_ps, mybir.ActivationFunctionType.Sigmoid
                    )
                    # st = (z > cut) * sigmoid(z), both packed sub-blocks
                    st_sb = sgp.tile([128, BLK], F16)
                    nc.vector.scalar_tensor_tensor(
                        out=st_sb,
                        in0=z_ps,
                        scalar=SIG_CUT,
                        in1=sig_sb,
                        op0=AluOpType.is_gt,
                        op1=AluOpType.mult,
                    )

                    # second matmul / copy / store run one pair behind so
                    # the PE never waits on the current pair's sigmoid/STT
                    # (depth 2 measured worse: 100.0us vs 98.4us)
                    pends.append((st_sb, x_mb, o_mb, base, lp, sz, rowoff))
                    if len(pends) > 1:
                        flush(pends.pop(0))
                base += sz

            for p in pends:
                flush(p)

    nc.compile()
    return nc


def plan_permutation(s_mti):
    """Row permutation grouping keep-rows first (no padding).
    Returns (perm, cut) with cut = number of keep rows."""
    n_mask = np.ones(N, np.float32)
    n_mask[np.asarray(s_mti)] = 0.0
    t_mask = np.ones(P, np.float32)
    t_mask[T_MTI] = 0.0
    keep = (n_mask[:, None] * t_mask[None, :]).reshape(-1)   # [NP]
    keep_core = np.tile(keep, BPC)                           # [BPC*NP]
    idx_keep = np.flatnonzero(keep_core == 1.0)
    idx_masked = np.flatnonzero(keep_core == 0.0)
    perm = np.concatenate([idx_keep, idx_masked])
    return perm, len(idx_keep)


def host_inputs(patches, u_prompt, m_prompt, s_mti):
    patches = np.asarray(patches, dtype=np.float32)
    u = np.asarray(u_prompt, dtype=np.float32)
    m = np.asarray(m_prompt, dtype=np.float32)

    C = np.concatenate([u, m], axis=0)                       # [128, 128]
    cf = C.astype(np.float16)
    ct = np.ascontiguousarray(cf.T)                          # [D, 2K] f16
    cboth = np.ascontiguousarray(
        np.concatenate([np.concatenate([cf[:K], cf[K:]], 1)] * 2, 0)
    )                                                        # [128, 256]
    ident = np.eye(D, dtype=np.float16)

    perm, cut = plan_permutation(s_mti)

    x_flat = patches.astype(np.float16).reshape(B, NP, D)
    in_maps = []
    for c in range(NCORES):
        xT = x_flat[c * BPC:(c + 1) * BPC].reshape(BPC * NP, D).T  # [D, rows]
        xs = np.ascontiguousarray(xT[:, perm])
        in_maps.append({"x": xs, "ct": ct, "cboth": cboth, "ident": ident})
    return in_maps, (perm, cut)


_NC_CACHE = {}


def kernel(patches, u_prompt, m_prompt, s_mti, s_uti=None, trace=False, **kw):
    from concourse.bass_utils import run_bass_kernel_spmd

    in_maps, (perm, cut) = host_inputs(patches, u_prompt, m_prompt, s_mti)

    if cut not in _NC_CACHE:
        _NC_CACHE[cut] = build_nc(cut)
    nc = _NC_CACHE[cut]

    res = run_bass_kernel_spmd(nc, in_maps, list(range(NCORES)), trace=trace)
    out = np.empty((B, NP, D), np.float32)
    for c in range(NCORES):
        oT = res.results[c]["out"]                           # [D, ROWS] f16
        dst = out[c * BPC:(c + 1) * BPC].reshape(BPC * NP, D)
        dst[perm] = oT.T
    out = out.reshape(B, N, P, D)
    if trace:
        kernel.last_results = res
    return out
